# revision 13
# baseline (speedup 1.0000x reference)
"""Trainium2 Bass kernel for nn_NeighborhoodSearch (sparse_attention).

Sharding: 8 cores = (batch b in {0,1}) x (head-pair hp in {0..3}); each core
computes a full-[N, D] partial contribution of its 2 heads through its slice
of Wo; the host sums the 4 partials per batch (and transposes back).

Math notes (validated against the reference in fp64):
 - The neighborhood "attention" softmax is over a singleton axis -> weights
   are all 1, so fused = sum of the 9 padded neighbors of x2 on the 48x48
   grid.  Padding slots replicate the first valid neighbor, which is
   (max(r-1,0), max(c-1,0)), with multiplicity 9 - vh(r)*vw(c).  This makes
   fused = boxsum3x3_zeropad(x2) + w2(r,c) * x2[max(r-1,0), max(c-1,0)]
   with w2 = 3*eh + 3*ew - eh*ew  (eh/ew = 1 at grid edges) in {0, 3, 5}.
 - LayerNorm feeds only the q projection, so it is folded into it:
   q = rstd[n] * (F @ (gamma*WqT*s)) - (rstd*mu)[n] * sum_d(gamma*WqT*s)
       + (beta @ WqT*s + bq*s)
 - Softmax needs no max-subtraction (scores are ~N(0, 0.3), |s| < ~3).
   The denominator comes free from an appended ones-column on v.

Device layout is feature-transposed ([D, N]); the host pre-transposes
x1/x2 and post-transposes the output.
"""

import os
import sys

sys.path.insert(0, "/opt/trn_rl_repo")

import numpy as np

import concourse.bass as bass
import concourse.mybir as mybir
import concourse.tile as tile
from concourse.bass_utils import run_bass_kernel_spmd
from concourse.masks import make_identity

# ---------------------------------------------------------------- constants
B = 2
N = 2304          # sequence length = 48*48
D = 768           # model dim
G = 48            # grid side
P = 128           # partitions
DC = D // P       # 6 feature chunks
HD = 96           # head dim
HPC = 2           # heads per core
NQB = 384         # n-block width (matmul moving free dim)
NB = N // NQB     # 6 n-blocks
NKC = N // P      # 18 key chunks
EPS = 1e-5
QSCALE = HD ** -0.5

F32 = mybir.dt.float32
F32R = mybir.dt.float32r  # fast fp32 matmul mode (1 cyc/row at free>=256);
# operands of f32r matmuls must be produced (rounded) as f32r, so every
# tensor feeding a matmul is declared f32r end-to-end (bit-identical to f32
# in DRAM / numpy).

ADD = mybir.AluOpType.add
SUB = mybir.AluOpType.subtract
MULT = mybir.AluOpType.mult
BYPASS = mybir.AluOpType.bypass
EXP = mybir.ActivationFunctionType.Exp
SQUARE = mybir.ActivationFunctionType.Square
SQRT = mybir.ActivationFunctionType.Sqrt


def _patch_tile_drain():
    """This container's walrus accepts at most 1 sync-wait per instruction
    (2 for EventSemaphore), but TileContext's final drain can carry several.
    Split the excess waits onto single-wait SP nops emitted after the drain
    (all complete before the all-engine barrier, so semantics are kept)."""
    if getattr(tile.TileContext, "_drain_patched", False):
        return
    from concourse.tile import ScopedClock

    def _drain_and_barrier(self, tick_clock, wait_clock):
        nc = self.nc
        drain_inst = nc.sync.drain()
        wait_clock.add_sem_waits(
            drain_inst.ins, ScopedClock({None: tick_clock.global_clock})
        )
        si = drain_inst.ins.sync_info
        waits = list(si.on_wait or [])
        if len(waits) > 1:
            si.on_wait = waits[:1]
            for w in waits[1:]:
                nop = nc.sync.nop(nofuse=True)
                nsi = nop.ins.sync_info
                if nsi is None:
                    nop.ins.sync_info = mybir.SyncInfo(on_wait=[w], on_update=[])
                else:
                    nsi.on_wait = (nsi.on_wait or []) + [w]
        nc.all_engine_barrier()
        popped = nc._tile_sem_poison_stack.pop()
        assert popped is self._sem_poison
        nc.clear_and_free_semaphores(list(self.sems.allocated().values()))
        nc.all_engine_barrier()

    tile.TileContext._drain_and_barrier = _drain_and_barrier
    tile.TileContext._drain_patched = True


def _split_multiwaits(nc):
    """This walrus supports at most 1 sync-wait per instruction; move excess
    waits onto single-wait NoOps inserted just before (same engine)."""
    for fn in nc.m.functions:
        for blk in fn.blocks:
            insts = list(blk.instructions)
            new = []
            changed = False
            for inst in insts:
                si = inst.sync_info
                if si is not None and si.on_wait and len(si.on_wait) > 1:
                    waits = list(si.on_wait)
                    for j, wcond in enumerate(waits[:-1]):
                        nop = mybir.InstNoOp(
                            name=f"{inst.name}-w{j}", engine=inst.engine,
                            ins=[], outs=[],
                            sync_info=mybir.SyncInfo(on_wait=[wcond],
                                                     on_update=[]))
                        new.append(nop)
                    si.on_wait = waits[-1:]
                    changed = True
                new.append(inst)
            if changed:
                blk.instructions = new


def build_nc(split_waits=True):
    _patch_tile_drain()
    nc = bass.Bass("TRN2", target_bir_lowering=False, debug=False)

    x1t = nc.dram_tensor("x1t", [D, N], F32R, kind="ExternalInput").ap()
    x2t = nc.dram_tensor("x2t", [D, N], F32R, kind="ExternalInput").ap()
    wk = nc.dram_tensor("wk", [D, HPC * HD], F32R, kind="ExternalInput").ap()
    wv = nc.dram_tensor("wv", [D, HPC * HD], F32R, kind="ExternalInput").ap()
    wqg = nc.dram_tensor("wqg", [D, HPC * HD], F32R, kind="ExternalInput").ap()
    wo = nc.dram_tensor("wo", [P, HPC, D], F32R, kind="ExternalInput").ap()
    vecs = nc.dram_tensor("vecs", [HPC * HD, 4], F32, kind="ExternalInput").ap()
    cst = nc.dram_tensor("cst", [P, 2], F32R, kind="ExternalInput").ap()
    bo = nc.dram_tensor("bo", [D], F32, kind="ExternalInput").ap()
    outp = nc.dram_tensor("outp", [D, N], F32, kind="ExternalOutput").ap()

    x1r = x1t.rearrange("(c p) n -> c p n", p=P)
    x2r = x2t.rearrange("(c p) n -> c p n", p=P)
    outr = outp.rearrange("(c p) n -> c p n", p=P)

    with tile.TileContext(nc) as tc:
        # -------------------------------------------------- persistent pools
        with tc.tile_pool(name="glob", bufs=1) as gp, \
             tc.tile_pool(name="dram", bufs=1, space="DRAM") as dp:
            # weights
            sb_wk = gp.tile([P, DC, HPC * HD], F32R, tag="wk")
            nc.sync.dma_start(sb_wk, wk.rearrange("(c p) q -> p c q", p=P))
            sb_wv = gp.tile([P, DC, HPC * HD], F32R, tag="wv")
            nc.sync.dma_start(sb_wv, wv.rearrange("(c p) q -> p c q", p=P))
            sb_wqg = gp.tile([P, DC, HPC * HD], F32R, tag="wqg")
            nc.sync.dma_start(sb_wqg, wqg.rearrange("(c p) q -> p c q", p=P))
            sb_wo = gp.tile([P, HPC, D], F32R, tag="wo")
            nc.sync.dma_start(sb_wo, wo)
            # small per-partition constants: col 0 ones, 1 zero, 2 eps,
            # 3+h bk, 5+h bv, 7+h negg, 9+h cq, 11+c bo
            sb_c = gp.tile([P, 20], F32, tag="consts")
            nc.vector.memset(sb_c, 0.0)
            nc.vector.memset(sb_c[:, 0:1], 1.0)
            nc.vector.memset(sb_c[:, 2:3], EPS)
            nc.gpsimd.dma_start(
                sb_c[0:HD, 3:11].rearrange("p (k h) -> p k h", h=HPC),
                vecs.rearrange("(h p) k -> p k h", p=HD))
            nc.gpsimd.dma_start(sb_c[:, 11:11 + DC],
                                bo.rearrange("(c p) -> p c", p=P))
            sb_cst = gp.tile([P, 2], F32R, tag="cst")
            nc.sync.dma_start(sb_cst, cst)
            ones = sb_cst[:, 0:1]
            zro = sb_cst[:, 1:2]
            eps_b = sb_c[:, 2:3]

            ident = gp.tile([P, P], F32, tag="ident")
            make_identity(nc, ident)

            # activations kept for attention
            sb_q = gp.tile([P, HPC, N], F32R, tag="qT")
            nc.vector.tensor_copy(
                sb_q[HD:P, :, :],
                zro[HD:P, :, None].to_broadcast([P - HD, HPC, N]))
            sb_k = gp.tile([P, HPC, N], F32R, tag="kT")
            nc.vector.tensor_copy(
                sb_k[HD:P, :, :],
                zro[HD:P, :, None].to_broadcast([P - HD, HPC, N]))
            sb_v = gp.tile([P, HPC, NKC, HD + 1], F32R, tag="vnat")
            nc.vector.tensor_copy(
                sb_v[:, :, :, HD:HD + 1],
                ones[:, :, None, None].to_broadcast([P, HPC, NKC, 1]))

            dstats = dp.tile([2, N], F32, tag="dstats")
            ddens = []
            for i in range(4):
                dden_t = dp.tile([1, NQB], F32, tag=f"dden{i}", name=f"dden{i}")
                ddens.append(dden_t)

            # ------------------------------------------------ phase 1 + LN
            with tc.tile_pool(name="fpool", bufs=1) as fp:
                f_tiles = []
                stats = fp.tile([P, 2 * N], F32, tag="stats")

                with tc.tile_pool(name="p1", bufs=1) as p1, \
                     tc.tile_pool(name="x2p", bufs=2) as x2p, \
                     tc.tile_pool(name="x1p", bufs=2) as x1p, \
                     tc.tile_pool(name="vbp", bufs=3) as vbp, \
                     tc.tile_pool(name="sqp", bufs=2) as sqp, \
                     tc.tile_pool(name="stg", bufs=2) as stg, \
                     tc.tile_pool(name="ppj", bufs=1, space="PSUM") as ppj, \
                     tc.tile_pool(name="pps", bufs=1, space="PSUM") as pps, \
                     tc.tile_pool(name="ppt", bufs=2, space="PSUM") as ppt, \
                     tc.tile_pool(name="ppq", bufs=1, space="PSUM") as ppq:

                    # ---- neighborhood sums (DVE) per feature chunk
                    for c in range(DC):
                        x = x2p.tile([P, N], F32R, tag="x2")
                        nc.sync.dma_start(x, x2r[c])
                        fc = fp.tile([P, N], F32R, tag=f"f{c}")
                        f_tiles.append(fc)
                        cc = p1.tile([P, N], F32R, tag="ctmp")
                        # column (c-direction) 3-sum with zero edges
                        nc.vector.tensor_tensor(cc[:, 0:N - 1], x[:, 0:N - 1],
                                                x[:, 1:N], op=ADD)
                        nc.vector.tensor_copy(cc[:, N - 1:N], x[:, N - 1:N])
                        nc.vector.tensor_tensor(cc[:, 1:N], cc[:, 1:N],
                                                x[:, 0:N - 1], op=ADD)
                        c3 = cc.rearrange("p (r g) -> p r g", g=G)
                        x3 = x.rearrange("p (r g) -> p r g", g=G)
                        # undo the wrap-around terms at the row seams
                        nc.vector.tensor_tensor(c3[:, 1:G, 0:1], c3[:, 1:G, 0:1],
                                                x3[:, 0:G - 1, G - 1:G], op=SUB)
                        nc.vector.tensor_tensor(c3[:, 0:G - 1, G - 1:G],
                                                c3[:, 0:G - 1, G - 1:G],
                                                x3[:, 1:G, 0:1], op=SUB)
                        # row (r-direction) 3-sum with zero edges
                        nc.vector.tensor_tensor(fc[:, 0:N - G], cc[:, 0:N - G],
                                                cc[:, G:N], op=ADD)
                        nc.vector.tensor_copy(fc[:, N - G:N], cc[:, N - G:N])
                        nc.vector.tensor_tensor(fc[:, G:N], fc[:, G:N],
                                                cc[:, 0:N - G], op=ADD)
                        # border corrections: F += w2 * x[max(r-1,0), max(c-1,0)]
                        f3 = fc.rearrange("p (r g) -> p r g", g=G)
                        stt = nc.vector.scalar_tensor_tensor
                        # top row r=0, c=1..47 (+3)
                        stt(f3[:, 0, 1:G], x3[:, 0, 0:G - 1], 3.0,
                            f3[:, 0, 1:G], op0=MULT, op1=ADD)
                        # bottom row r=47, c=1..47 (+3)
                        stt(f3[:, G - 1, 1:G], x3[:, G - 2, 0:G - 1], 3.0,
                            f3[:, G - 1, 1:G], op0=MULT, op1=ADD)
                        # left col c=0, r=1..47 (+3)
                        stt(f3[:, 1:G, 0:1], x3[:, 0:G - 1, 0:1], 3.0,
                            f3[:, 1:G, 0:1], op0=MULT, op1=ADD)
                        # right col c=47, r=1..46 (+3)
                        stt(f3[:, 1:G - 1, G - 1:G], x3[:, 0:G - 2, G - 2:G - 1],
                            3.0, f3[:, 1:G - 1, G - 1:G], op0=MULT, op1=ADD)
                        # corners: (0,0) +5; (0,47) +2; (47,0) +2; (47,47) +2
                        stt(f3[:, 0, 0:1], x3[:, 0, 0:1], 5.0,
                            f3[:, 0, 0:1], op0=MULT, op1=ADD)
                        stt(f3[:, 0, G - 1:G], x3[:, 0, G - 2:G - 1], 2.0,
                            f3[:, 0, G - 1:G], op0=MULT, op1=ADD)
                        stt(f3[:, G - 1, 0:1], x3[:, G - 2, 0:1], 2.0,
                            f3[:, G - 1, 0:1], op0=MULT, op1=ADD)
                        stt(f3[:, G - 1, G - 1:G], x3[:, G - 2, G - 2:G - 1], 2.0,
                            f3[:, G - 1, G - 1:G], op0=MULT, op1=ADD)

                    # ---- k/v projections from streamed x1 blocks (PE)
                    for nb in range(NB):
                        ns = slice(nb * NQB, (nb + 1) * NQB)
                        xb = x1p.tile([P, DC, NQB], F32R, tag="x1b")
                        for c in range(DC):
                            nc.sync.dma_start(xb[:, c, :], x1r[c][:, ns])
                        for h in range(HPC):
                            hs = slice(h * HD, (h + 1) * HD)
                            psk = ppj.tile([HD, NQB], F32, tag="pk")
                            psv = ppj.tile([HD, NQB], F32, tag="pv")
                            for c in range(DC):
                                nc.tensor.matmul(psk, (sb_wk[:, c, hs]),
                                                 (xb[:, c, :]),
                                                 start=(c == 0), stop=(c == DC - 1))
                            for c in range(DC):
                                nc.tensor.matmul(psv, (sb_wv[:, c, hs]),
                                                 (xb[:, c, :]),
                                                 start=(c == 0), stop=(c == DC - 1))
                            nc.vector.tensor_scalar(sb_k[0:HD, h, ns], psk,
                                                    sb_c[0:HD, 3 + h:4 + h], None,
                                                    op0=ADD)
                            vb = vbp.tile([P, NQB], F32, tag="vblk")
                            nc.vector.memset(vb[HD:P, :], 0.0)
                            nc.vector.tensor_scalar(vb[0:HD, :], psv,
                                                    sb_c[0:HD, 5 + h:6 + h], None,
                                                    op0=ADD)
                            # transpose v into natural [nk, hd] layout
                            for t in range(NQB // P):
                                kc = nb * (NQB // P) + t
                                pst = ppt.tile([P, P], F32, tag="ptr")
                                nc.tensor.transpose(pst, vb[:, t * P:(t + 1) * P],
                                                    ident)
                                nc.scalar.copy(sb_v[:, h, kc, 0:HD], pst[:, 0:HD])

                    # ---- LN statistics (sum and sum-of-squares over d)
                    for nb in range(NB):
                        ns = slice(nb * NQB, (nb + 1) * NQB)
                        psx = pps.tile([1, NQB], F32, tag="psx")
                        psq = pps.tile([1, NQB], F32, tag="psq")
                        for c in range(DC):
                            sq = sqp.tile([P, NQB], F32R, tag="sq")
                            nc.scalar.activation(sq, f_tiles[c][:, ns], SQUARE)
                            nc.tensor.matmul(psx, (ones), (f_tiles[c][:, ns]),
                                             start=(c == 0), stop=(c == DC - 1))
                            nc.tensor.matmul(psq, (ones), (sq),
                                             start=(c == 0), stop=(c == DC - 1))
                        s1 = stg.tile([1, NQB], F32, tag="stg")
                        nc.scalar.copy(s1, psx)
                        nc.sync.dma_start(dstats[0:1, ns], s1)
                        s2 = stg.tile([1, NQB], F32, tag="stg")
                        nc.scalar.copy(s2, psq)
                        nc.sync.dma_start(dstats[1:2, ns], s2)

                    # ---- broadcast stats to all partitions, compute
                    #      a = rstd, b = mu * rstd (in place in `stats`)
                    nc.gpsimd.dma_start(
                        stats, dstats.rearrange("a n -> (a n)")[None, :]
                        .to_broadcast([P, 2 * N]))
                    mu = stats[:, 0:N]
                    vr = stats[:, N:2 * N]
                    nc.vector.tensor_scalar(mu, mu, 1.0 / D, None, op0=MULT)
                    nc.vector.tensor_scalar(vr, vr, 1.0 / D, None, op0=MULT)
                    musq = p1.tile([P, N], F32, tag="ctmp")
                    nc.vector.tensor_mul(musq, mu, mu)
                    nc.vector.tensor_tensor(vr, vr, musq, op=SUB)
                    nc.scalar.activation(vr, vr, SQRT, bias=eps_b)
                    nc.vector.reciprocal(vr, vr)          # a = rstd
                    nc.vector.tensor_tensor(mu, mu, vr, op=MULT)  # b = mu*rstd

                    # ---- q projection with LN folded in
                    for nb in range(NB):
                        ns = slice(nb * NQB, (nb + 1) * NQB)
                        for h in range(HPC):
                            hs = slice(h * HD, (h + 1) * HD)
                            psq2 = ppq.tile([HD, NQB], F32, tag="pq")
                            for c in range(DC):
                                nc.tensor.matmul(psq2, (sb_wqg[:, c, hs]),
                                                 (f_tiles[c][:, ns]),
                                                 start=(c == 0), stop=(c == DC - 1))
                            qsl = sb_q[0:HD, h, ns]
                            nc.vector.tensor_tensor(qsl, psq2, vr[0:HD, ns],
                                                    op=MULT)
                            nc.vector.scalar_tensor_tensor(
                                qsl, mu[0:HD, ns], sb_c[0:HD, 7 + h:8 + h], qsl,
                                op0=MULT, op1=ADD)
                            nc.vector.tensor_scalar(qsl, qsl,
                                                    sb_c[0:HD, 9 + h:10 + h], None,
                                                    op0=ADD)

            # ---------------------------------------------------- attention
            with tc.tile_pool(name="att", bufs=2) as ap_, \
                 tc.tile_pool(name="ot", bufs=1) as otp, \
                 tc.tile_pool(name="den", bufs=2) as dnp, \
                 tc.tile_pool(name="ost", bufs=3) as osp, \
                 tc.tile_pool(name="ppk", bufs=3, space="PSUM") as ppk, \
                 tc.tile_pool(name="ppa", bufs=2, space="PSUM") as ppa, \
                 tc.tile_pool(name="ppw", bufs=2, space="PSUM") as ppw:

                sb_o = otp.tile([P, HPC, N], F32R, tag="oT")
                nc.vector.tensor_copy(
                    sb_o[HD:P, :, :],
                    zro[HD:P, :, None].to_broadcast([P - HD, HPC, N]))

                for nb in range(NB):
                    ns = slice(nb * NQB, (nb + 1) * NQB)
                    for h in range(HPC):
                        att = ap_.tile([P, NKC, NQB], F32R, tag="attT")
                        for kc in range(NKC):
                            ps = ppk.tile([P, NQB], F32, tag="ps")
                            nc.tensor.matmul(
                                ps, (sb_k[:, h, kc * P:(kc + 1) * P]),
                                (sb_q[:, h, ns]), start=True, stop=True)
                            nc.scalar.activation(att[:, kc, :], ps, EXP)
                        po = ppa.tile([HD + 1, NQB], F32, tag="po")
                        for kc in range(NKC):
                            nc.tensor.matmul(po, (sb_v[:, h, kc, :]),
                                             (att[:, kc, :]),
                                             start=(kc == 0), stop=(kc == NKC - 1))
                        d1 = dnp.tile([1, NQB], F32, tag="d1")
                        nc.scalar.copy(d1, po[HD:HD + 1, :])
                        dden = ddens[(nb * HPC + h) % 4]
                        nc.sync.dma_start(dden, d1)
                        dr = dnp.tile([HD, NQB], F32, tag="dr")
                        nc.gpsimd.dma_start(dr, dden.to_broadcast([HD, NQB]))
                        nc.vector.reciprocal(dr, dr)
                        nc.vector.tensor_tensor(sb_o[0:HD, h, ns], po[0:HD, :],
                                                dr, op=MULT)
                    # output projection for this n-block
                    for dc in range(DC):
                        pw = ppw.tile([P, NQB], F32, tag="pw")
                        for h in range(HPC):
                            nc.tensor.matmul(pw,
                                             (sb_wo[:, h, dc * P:(dc + 1) * P]),
                                             (sb_o[:, h, ns]),
                                             start=(h == 0), stop=(h == HPC - 1))
                        so = osp.tile([P, NQB], F32, tag="so")
                        nc.vector.tensor_scalar(so, pw,
                                                sb_c[:, 11 + dc:12 + dc], None,
                                                op0=ADD)
                        nc.sync.dma_start(outr[dc][:, ns], so)
    if split_waits:
        _split_multiwaits(nc)
    return nc


def make_core_inputs(inputs):
    """Host-side shard prep: slice/transpose weights, fold LN + q-scale."""
    x1 = np.ascontiguousarray(np.asarray(inputs["x1"], np.float32))
    x2 = np.ascontiguousarray(np.asarray(inputs["x2"], np.float32))
    WqT = np.asarray(inputs["Wq"], np.float32).T
    WkT = np.asarray(inputs["Wk"], np.float32).T
    WvT = np.asarray(inputs["Wv"], np.float32).T
    WoT = np.asarray(inputs["Wo"], np.float32).T
    bq = np.asarray(inputs["bq"], np.float32)
    bk = np.asarray(inputs["bk"], np.float32)
    bv = np.asarray(inputs["bv"], np.float32)
    bo = np.asarray(inputs["bo"], np.float32)
    gamma = np.asarray(inputs["ln_gamma"], np.float32)
    beta = np.asarray(inputs["ln_beta"], np.float32)

    x1t = [np.ascontiguousarray(x1[b].T) for b in range(B)]
    x2t = [np.ascontiguousarray(x2[b].T) for b in range(B)]

    cst_arr = np.zeros((P, 2), np.float32)
    cst_arr[:, 0] = 1.0
    in_maps = []
    for core in range(8):
        b, hp = divmod(core, 4)
        sl = slice(HPC * HD * hp, HPC * HD * (hp + 1))
        wq_s = (WqT[:, sl] * QSCALE).astype(np.float32)
        wqg = (gamma[:, None] * wq_s).astype(np.float32)
        negg = (-wqg.sum(axis=0)).astype(np.float32)
        cq = (beta @ wq_s + bq[sl] * QSCALE).astype(np.float32)
        vecs = np.stack([bk[sl], bv[sl], negg, cq], axis=1)  # [192, 4]
        wo_pad = np.zeros((P, HPC, D), np.float32)
        wo_pad[0:HD] = WoT[sl, :].reshape(HPC, HD, D).transpose(1, 0, 2)
        in_maps.append({
            "x1t": x1t[b],
            "x2t": x2t[b],
            "wk": np.ascontiguousarray(WkT[:, sl]),
            "wv": np.ascontiguousarray(WvT[:, sl]),
            "wqg": np.ascontiguousarray(wqg),
            "wo": wo_pad,
            "vecs": np.ascontiguousarray(vecs),
            "bo": bo if hp == 0 else np.zeros_like(bo),
            "cst": cst_arr,
        })
    return in_maps


def kernel(**inputs):
    in_maps = make_core_inputs(inputs)
    nc = build_nc()
    trace = os.environ.get("BASS_KERNEL_TRACE") == "1"
    res = run_bass_kernel_spmd(nc, in_maps, core_ids=list(range(8)),
                               trace=trace)
    if trace:
        kernel.last_exec_time_ns = res.exec_time_ns
        kernel.last_results = res
    outs = [r["outp"] for r in res.results]
    out = np.empty((B, N, D), np.float32)
    for b in range(B):
        acc = outs[4 * b] + outs[4 * b + 1]
        acc += outs[4 * b + 2]
        acc += outs[4 * b + 3]
        out[b] = acc.T
    return out
